# revision 6
# baseline (speedup 1.0000x reference)
"""Trainium2 Bass kernel for nn_FEASAI (refocus / depth-from-flow module).

Strategy (8 NeuronCores, SPMD shared program, per-core data differs):
  core c -> batch b = c//2, half = c%2. Each half-core handles:
    - 32 of the 64 voxelgrid time-slices (warp + accumulate)
    - 14 of the 27 occ/depth slices (27 padded to 2*14 with a zeroed dup)
    - gain-gated single-frame outputs (ev/img/gt depth frames)
  Host adds the per-pair partial sums and assembles [4, 6, 256, 256].

Warp = per-pixel horizontal bilinear resample with displacement bounded by
~1 pixel (flow in [EPS, 1+EPS), |t - reft| < 1).  out[x] =
sum_d hat(R[x] - d) * img[x + d], hat(z) = relu(1 - |z|), with
R = relative sample position; reference clipping semantics are reproduced
exactly by R = min(max(r, frac(r) - x), 255 - x), which differs from r
only at columns {0,1,254,255} (fixed with tiny border ops).

Slice layout on chip: [256,256] -> [128, 512] (partition p holds rows p and
p+128), padded to [128, 2*pad+512] so shifted-tap access patterns stay in
bounds.  Weights on ScalarE, products on VectorE, accumulation over slices
via TensorE identity-matmul into PSUM (start/stop accumulate flags).
"""
import numpy as np
import concourse.bacc as bacc
import concourse.bass as bass
import concourse.mybir as mybir
from concourse.tile import TileContext
from concourse.bass_utils import run_bass_kernel_spmd

EPS = 1e-3
BS, TS, TJ, H, W = 4, 64, 27, 256, 256
N_CORES = 8
TV = TS // 2          # voxel slices per core
JI = 14               # img slices per core (27 -> 14+13, half1 dup zeroed)
F = 512               # packed free dim: [128, 512] per [256,256] slice
FDT = mybir.dt.float32


def _pk(a):
    """[256,256] -> [128,512] packed: col blk*256+x on partition p = row blk*128+p."""
    return a.reshape(2, 128, 256).transpose(1, 0, 2).reshape(128, 512)


def _unpk(a):
    return a.reshape(128, 2, 256).transpose(1, 0, 2).reshape(256, 256)


def _dram_packed(t, i):
    """3-D AP for slice i of DRAM tensor t [N,256,256]: [p, blk, x]."""
    return t[i].rearrange("(blk p) x -> p blk x", blk=2)


def _sb_packed(tile_ap):
    """View a [128, 512] SBUF region as [p, blk, x]."""
    return tile_ap.rearrange("p (blk x) -> p blk x", blk=2)


def build(taps3: bool):
    nc = bacc.Bacc(None, target_bir_lowering=False, debug=False)
    dt = mybir.dt
    A = mybir.AluOpType
    AF = mybir.ActivationFunctionType

    for val in (-2.0, -1.0, 2.0):
        t = nc.alloc_sbuf_tensor(f"constx-{val}", [128, 1], mybir.dt.float32)
        nc.gpsimd.memset(t.ap(), val)
        nc.const_aps.aps[(mybir.dt.float32, val)] = t.ap()
    nc.all_engine_barrier()

    vox = nc.declare_dram_parameter("vox", [TV, H, W], FDT, isOutput=False)
    flowe = nc.declare_dram_parameter("flowe", [TV, H, W], FDT, isOutput=False)
    occ = nc.declare_dram_parameter("occ", [JI, H, W], FDT, isOutput=False)
    flowi = nc.declare_dram_parameter("flowi", [JI, H, W], FDT, isOutput=False)
    sfe = nc.declare_dram_parameter("sfe", [H, W], FDT, isOutput=False)
    sfi = nc.declare_dram_parameter("sfi", [H, W], FDT, isOutput=False)
    sdg = nc.declare_dram_parameter("sdg", [H, W], FDT, isOutput=False)
    # scal columns: [0:TV) -s_ev | [TV:TV+JI) -s_img | [TV+JI:TV+2JI) k_img gain
    #               | TV+2JI k_ev | TV+2JI+1 k_imgsingle | TV+2JI+2 g_gt
    NS = TV + 2 * JI + 3
    scal = nc.declare_dram_parameter("scal", [128, NS], FDT, isOutput=False)

    ov = nc.declare_dram_parameter("ov", [128, F], FDT, isOutput=True)
    oi = nc.declare_dram_parameter("oi", [128, F], FDT, isOutput=True)
    od = nc.declare_dram_parameter("od", [128, F], FDT, isOutput=True)
    oev = nc.declare_dram_parameter("oev", [128, F], FDT, isOutput=True)
    oiv = nc.declare_dram_parameter("oiv", [128, F], FDT, isOutput=True)
    ogt = nc.declare_dram_parameter("ogt", [128, F], FDT, isOutput=True)

    PAD = 1 if taps3 else 2
    FP = F + 2 * PAD          # padded tile width
    ds = (-1, 0, 1) if taps3 else (-2, -1, 0, 1, 2)

    with TileContext(nc) as tc:
        with tc.tile_pool(name="const", bufs=1) as cpool, \
             tc.tile_pool(name="io", bufs=3) as iop, \
             tc.tile_pool(name="wk", bufs=2) as wk, \
             tc.tile_pool(name="ps", bufs=1, space="PSUM") as psp:

            st = cpool.tile([128, NS], FDT, tag="st")
            nc.sync.dma_start(out=st[:], in_=scal[:])
            ident = cpool.tile([128, 128], FDT, tag="ident")
            iotap = cpool.tile([128, 1], FDT, tag="iotap")
            iotaf = cpool.tile([128, 128], FDT, tag="iotaf")
            nc.gpsimd.iota(iotap[:], pattern=[[0, 1]], channel_multiplier=1,
                           allow_small_or_imprecise_dtypes=True)
            nc.gpsimd.iota(iotaf[:], pattern=[[1, 128]], channel_multiplier=0,
                           allow_small_or_imprecise_dtypes=True)
            nc.vector.tensor_scalar(ident[:], iotaf[:], iotap[:, 0:1], None, A.is_equal)

            # border x-values tile: [0,1,0,1,0]; left AP -> [0,1,0,1], right AP -> [1,0,1,0]
            cb = cpool.tile([128, 5], FDT, tag="cb")
            nc.gpsimd.memset(cb[:], 0.0)
            nc.gpsimd.memset(cb[:, 1:4:2], 1.0)

            psv = psp.tile([128, F], FDT, tag="psv")
            psi = psp.tile([128, F], FDT, tag="psi")
            psd = psp.tile([128, F], FDT, tag="psd")

            def border_fix(r):
                """In-place: r <- min(max(r, frac(r)-x), 255-x) on border cols."""
                rc = r[:, PAD:PAD + F].rearrange("p (blk x) -> p blk x", blk=2)
                rl = rc[:, :, 0:2]
                rr = rc[:, :, 254:256]
                cbl = cb[:, 0:4].rearrange("p (blk x) -> p blk x", blk=2)   # x
                cbr = cb[:, 1:5].rearrange("p (blk x) -> p blk x", blk=2)   # 255-x
                ib = wk.tile([128, 2, 2], dt.int32, tag="ib")
                fb = wk.tile([128, 2, 2], FDT, tag="fb")
                wb = wk.tile([128, 2, 2], FDT, tag="wb")
                nc.vector.tensor_scalar(fb[:], rl, 0.5, None, A.subtract)
                nc.vector.tensor_copy(ib[:], fb[:])          # rint -> floor(r)
                nc.vector.tensor_copy(fb[:], ib[:])
                nc.vector.tensor_tensor(wb[:], rl, fb[:], A.subtract)   # frac
                nc.vector.tensor_tensor(wb[:], wb[:], cbl, A.subtract)  # - x
                nc.vector.tensor_tensor(rl, rl, wb[:], A.max)
                nc.vector.tensor_tensor(rr, rr, cbr, A.min)

            def warp_mac(r, src, psum, gain_col, first, last):
                """psum += sum_d hat(r-d)*src_shift_d; optional per-slice gain on weights."""
                hs = {}
                if taps3:
                    z = wk.tile([128, F], FDT, tag="z")
                    nc.scalar.activation(z[:], r[:, PAD:PAD + F], AF.Abs)
                    h0 = wk.tile([128, F], FDT, tag="h0")
                    nc.scalar.activation(h0[:], z[:], AF.Copy, bias=1.0, scale=-1.0)
                    hs[0] = h0
                    hp = wk.tile([128, F], FDT, tag="hp")
                    nc.scalar.activation(hp[:], r[:, PAD:PAD + F], AF.Relu)
                    hs[1] = hp
                    hm = wk.tile([128, F], FDT, tag="hm")
                    nc.scalar.activation(hm[:], r[:, PAD:PAD + F], AF.Relu, scale=-1.0)
                    hs[-1] = hm
                else:
                    for d in ds:
                        z = wk.tile([128, F], FDT, tag=f"z{d}")
                        nc.scalar.activation(z[:], r[:, PAD:PAD + F], AF.Abs, bias=float(-d))
                        h = wk.tile([128, F], FDT, tag=f"h{d}")
                        nc.scalar.activation(h[:], z[:], AF.Relu, bias=1.0, scale=-1.0)
                        hs[d] = h
                for k, d in enumerate(ds):
                    p = wk.tile([128, F], FDT, tag=f"p{d}")
                    nc.vector.tensor_tensor(p[:], hs[d][:], src[:, PAD + d:PAD + d + F],
                                            A.mult)
                    nc.tensor.matmul(psum[:], ident[:], p[:],
                                     start=(first and k == 0), stop=(last and k == len(ds) - 1))

            # ---------------- voxel stream ----------------
            for t in range(TV):
                ft = iop.tile([128, FP], FDT, tag="ft")
                nc.sync.dma_start(out=_sb_packed(ft[:, PAD:PAD + F]), in_=_dram_packed(flowe, t))
                vt = iop.tile([128, FP], FDT, tag="vt")
                nc.sync.dma_start(out=_sb_packed(vt[:, PAD:PAD + F]), in_=_dram_packed(vox, t))
                # finite pads (weights there are exactly 0)
                nc.gpsimd.memset(vt[:, 0:PAD], 0.0)
                nc.gpsimd.memset(vt[:, PAD + F:], 0.0)

                r = wk.tile([128, FP], FDT, tag="r")
                nc.vector.tensor_scalar(r[:, PAD:PAD + F], ft[:, PAD:PAD + F],
                                        EPS, st[:, t:t + 1], A.add, A.mult)
                border_fix(r)
                warp_mac(r, vt, psv, None, first=(t == 0), last=(t == TV - 1))

            # ---------------- img + depth stream ----------------
            for j in range(JI):
                ft = iop.tile([128, FP], FDT, tag="ft")
                nc.sync.dma_start(out=_sb_packed(ft[:, PAD:PAD + F]), in_=_dram_packed(flowi, j))
                ot = iop.tile([128, FP], FDT, tag="vt")
                nc.sync.dma_start(out=_sb_packed(ot[:, PAD:PAD + F]), in_=_dram_packed(occ, j))
                nc.gpsimd.memset(ot[:, 0:PAD], 0.0)
                nc.gpsimd.memset(ot[:, PAD + F:], 0.0)

                fp = wk.tile([128, FP], FDT, tag="fp")
                nc.vector.tensor_scalar(fp[:, PAD:PAD + F], ft[:, PAD:PAD + F],
                                        EPS, None, A.add)
                r = wk.tile([128, FP], FDT, tag="r")
                nc.vector.tensor_scalar(r[:, PAD:PAD + F], fp[:, PAD:PAD + F],
                                        st[:, TV + j:TV + j + 1], None, A.mult)
                border_fix(r)

                dep = wk.tile([128, FP], FDT, tag="dep")
                nc.vector.reciprocal(dep[:, PAD:PAD + F], fp[:, PAD:PAD + F])
                nc.vector.tensor_scalar(dep[:, PAD:PAD + F], dep[:, PAD:PAD + F],
                                        st[:, TV + JI + j:TV + JI + j + 1], None, A.mult)
                nc.gpsimd.memset(dep[:, 0:PAD], 0.0)
                nc.gpsimd.memset(dep[:, PAD + F:], 0.0)

                warp_mac(r, ot, psi, None, first=(j == 0), last=(j == JI - 1))
                warp_mac(r, dep, psd, None, first=(j == 0), last=(j == JI - 1))

            # ---------------- singles ----------------
            def single_recip(src_dram, gain_col, out_dram):
                t_in = iop.tile([128, F], FDT, tag="sing")
                nc.sync.dma_start(out=_sb_packed(t_in[:]),
                                  in_=src_dram.rearrange("(blk p) x -> p blk x", blk=2))
                t2 = wk.tile([128, F], FDT, tag="sing2")
                nc.vector.tensor_scalar(t2[:], t_in[:], EPS, None, A.add)
                nc.vector.reciprocal(t2[:], t2[:])
                nc.vector.tensor_scalar(t2[:], t2[:], st[:, gain_col:gain_col + 1],
                                        None, A.mult)
                nc.sync.dma_start(out=out_dram[:], in_=t2[:])

            single_recip(sfe, TV + 2 * JI, oev)
            single_recip(sfi, TV + 2 * JI + 1, oiv)
            tgt = iop.tile([128, F], FDT, tag="sing")
            nc.sync.dma_start(out=_sb_packed(tgt[:]),
                              in_=sdg.rearrange("(blk p) x -> p blk x", blk=2))
            tg2 = wk.tile([128, F], FDT, tag="sing2")
            nc.vector.tensor_scalar(tg2[:], tgt[:], st[:, TV + 2 * JI + 2:TV + 2 * JI + 3],
                                    None, A.mult)
            nc.sync.dma_start(out=ogt[:], in_=tg2[:])

            # ---------------- psum -> out ----------------
            for psum, out_dram, scale in ((psv, ov, 1.0 / TS), (psi, oi, 1.0 / TJ),
                                          (psd, od, 1.0 / TJ)):
                o = wk.tile([128, F], FDT, tag="ocp")
                nc.scalar.activation(o[:], psum[:], AF.Copy, bias=0.0, scale=scale)
                nc.sync.dma_start(out=out_dram[:], in_=o[:])

    nc.finalize()
    return nc


_CACHED = {}
_RUNNERS = {}
LAST_EXEC_NS = None


def _build_runner(nc, n_cores=N_CORES):
    """Compiled SPMD callable mirroring bass2jax.run_bass_via_pjrt (no donation)."""
    import jax
    import numpy as _np
    from jax.sharding import Mesh, PartitionSpec
    try:
        from jax.experimental.shard_map import shard_map
    except ImportError:
        from jax.shard_map import shard_map
    from concourse import bass2jax, mybir as _mybir

    bass2jax.install_neuronx_cc_hook()
    partition_name = nc.partition_id_tensor.name if nc.partition_id_tensor else None
    in_names, out_names, out_avals, zero_outs = [], [], [], []
    for alloc in nc.m.functions[0].allocations:
        if not isinstance(alloc, _mybir.MemoryLocationSet):
            continue
        name = alloc.memorylocations[0].name
        if alloc.kind == "ExternalInput":
            if name != partition_name:
                in_names.append(name)
        elif alloc.kind == "ExternalOutput":
            shape = tuple(alloc.tensor_shape)
            dtype = _mybir.dt.np(alloc.dtype)
            out_names.append(name)
            out_avals.append(jax.core.ShapedArray(shape, dtype))
            zero_outs.append(_np.zeros(shape, dtype))
    n_params = len(in_names)
    all_in_names = in_names + out_names
    if partition_name is not None:
        all_in_names = all_in_names + [partition_name]

    def _body(*args):
        operands = list(args)
        if partition_name is not None:
            operands.append(bass2jax.partition_id_tensor())
        outs = bass2jax._bass_exec_p.bind(
            *operands,
            out_avals=tuple(out_avals),
            in_names=tuple(all_in_names),
            out_names=tuple(out_names),
            lowering_input_output_aliases=(),
            sim_require_finite=True,
            sim_require_nnan=True,
            nc=nc,
        )
        return tuple(outs)

    devices = jax.devices()[:n_cores]
    mesh = Mesh(np.asarray(devices), ("core",))
    in_specs = (PartitionSpec("core"),) * (n_params + len(out_names))
    out_specs = (PartitionSpec("core"),) * len(out_names)
    sharded = jax.jit(shard_map(_body, mesh=mesh, in_specs=in_specs,
                                out_specs=out_specs, check_rep=False))

    def run(in_maps, time_iters=0):
        concat_in = [np.concatenate([np.asarray(m[name]) for m in in_maps], axis=0)
                     for name in in_names]
        concat_zeros = [np.concatenate([z] * n_cores, axis=0) for z in zero_outs]
        dev_args = [jax.device_put(a) for a in concat_in + concat_zeros]
        outs = sharded(*dev_args)
        jax.block_until_ready(outs)
        exec_ns = None
        if time_iters:
            import time as _t
            best = float("inf")
            for _ in range(time_iters):
                t0 = _t.perf_counter()
                outs = sharded(*dev_args)
                jax.block_until_ready(outs)
                best = min(best, _t.perf_counter() - t0)
            exec_ns = int(best * 1e9)
        host_outs = [np.asarray(o) for o in outs]
        results = []
        for c in range(n_cores):
            d = {}
            for name, arr in zip(out_names, host_outs):
                per = arr.shape[0] // n_cores
                d[name] = arr[c * per:(c + 1) * per]
            results.append(d)
        return results, exec_ns

    return run


def _get_nc(taps3: bool):
    if taps3 not in _CACHED:
        _CACHED[taps3] = build(taps3)
    return _CACHED[taps3]


def kernel(voxelgrid, time, occ_aps, occ_t, gt_t, fx, v, depth_gt, flow_27):
    voxelgrid = np.asarray(voxelgrid, dtype=np.float32)
    time = np.asarray(time, dtype=np.float32)
    occ_aps = np.asarray(occ_aps, dtype=np.float32)
    occ_t = np.asarray(occ_t, dtype=np.float32)
    gt_t = np.asarray(gt_t, dtype=np.float32)
    fx = np.asarray(fx, dtype=np.float32)
    v = np.asarray(v, dtype=np.float32)
    depth_gt = np.asarray(depth_gt, dtype=np.float32)
    flow_27 = np.asarray(flow_27, dtype=np.float32)

    s_ev = time - gt_t[:, None]                     # [4,64]
    s_img = occ_t - gt_t[:, None]                   # [4,27]
    k = fx[:, 0, 0] * np.abs(v)                     # [4] depth numerator
    dist = np.abs(occ_t[:, None, :] - time[:, :, None])
    idx = np.argmin(dist, axis=2)                   # [4,64]
    ev_idx = np.argmin(np.abs(s_ev), axis=1)        # [4]
    img_idx = np.argmin(np.abs(s_img), axis=1)      # [4]

    taps3 = float(np.max(np.abs(np.concatenate([s_ev.ravel(), s_img.ravel()])))) \
        * (1.0 + EPS) < 1.0

    NS = TV + 2 * JI + 3
    in_maps = []
    for c in range(N_CORES):
        b, half = c // 2, c % 2
        tlo = half * TV
        tsl = slice(tlo, tlo + TV)
        jlist = list(range(0, JI)) if half == 0 else list(range(JI, TJ)) + [TJ - 1]
        jdup = [False] * JI if half == 0 else [False] * (TJ - JI) + [True]

        vox_s = voxelgrid[b, tsl]                                   # [32,H,W]
        flowe_s = flow_27[b, idx[b, tlo:tlo + TV]]                  # [32,H,W]
        occ_s = np.stack([np.zeros((H, W), np.float32) if dup else occ_aps[b, j]
                          for j, dup in zip(jlist, jdup)])
        flowi_s = flow_27[b, jlist]

        scal = np.zeros((128, NS), np.float32)
        scal[:, 0:TV] = -s_ev[b, tsl][None, :]
        scal[:, TV:TV + JI] = -s_img[b, jlist][None, :]
        scal[:, TV + JI:TV + 2 * JI] = np.where(jdup, 0.0, k[b])[None, :]

        own_ev = (tlo <= ev_idx[b] < tlo + TV)
        own_img = img_idx[b] in [j for j, dup in zip(jlist, jdup) if not dup]
        sfe_s = flow_27[b, idx[b, ev_idx[b]]] if own_ev else np.ones((H, W), np.float32)
        sfi_s = flow_27[b, img_idx[b]] if own_img else np.ones((H, W), np.float32)
        sdg_s = depth_gt[b, img_idx[b]] if own_img else np.zeros((H, W), np.float32)
        scal[:, TV + 2 * JI] = k[b] if own_ev else 0.0
        scal[:, TV + 2 * JI + 1] = k[b] if own_img else 0.0
        scal[:, TV + 2 * JI + 2] = 1.0 if own_img else 0.0

        in_maps.append({
            "vox": np.ascontiguousarray(vox_s),
            "flowe": np.ascontiguousarray(flowe_s),
            "occ": np.ascontiguousarray(occ_s),
            "flowi": np.ascontiguousarray(flowi_s),
            "sfe": np.ascontiguousarray(sfe_s),
            "sfi": np.ascontiguousarray(sfi_s),
            "sdg": np.ascontiguousarray(sdg_s),
            "scal": scal,
        })

    import os
    nc = _get_nc(taps3)
    if taps3 not in _RUNNERS:
        _RUNNERS[taps3] = _build_runner(nc)
    iters = int(os.environ.get("KERNEL_TIME_ITERS", "0"))
    results, exec_ns = _RUNNERS[taps3](in_maps, time_iters=iters)
    global LAST_EXEC_NS
    LAST_EXEC_NS = exec_ns

    class _Res:
        pass
    res = _Res()
    res.results = results

    out = np.zeros((BS, 6, H, W), np.float32)
    for b in range(BS):
        r0, r1 = res.results[2 * b], res.results[2 * b + 1]
        out[b, 0] = _unpk(r0["ov"] + r1["ov"])
        out[b, 1] = _unpk(r0["oi"] + r1["oi"])
        out[b, 2] = _unpk(r0["od"] + r1["od"])
        out[b, 3] = _unpk(r0["oev"] + r1["oev"])
        out[b, 4] = _unpk(r0["oiv"] + r1["oiv"])
        out[b, 5] = _unpk(r0["ogt"] + r1["ogt"])
    return out


# revision 7
# speedup vs baseline: 1.2891x; 1.2891x over previous
"""Trainium2 Bass kernel for nn_FEASAI (refocus / depth-from-flow module).

Strategy (8 NeuronCores, SPMD shared program, per-core data differs):
  core c -> batch b = c//2, half = c%2. Each half-core handles:
    - 32 of the 64 voxelgrid time-slices (warp + accumulate)
    - 14 of the 27 occ/depth slices (27 padded to 2*14 with a zeroed dup)
    - gain-gated single-frame outputs (ev/img/gt depth frames)
  Host adds the per-pair partial sums and assembles [4, 6, 256, 256].

Warp = per-pixel horizontal bilinear resample with displacement bounded by
~1 pixel (flow in [EPS, 1+EPS), |t - reft| < 1).  out[x] =
sum_d hat(R[x] - d) * img[x + d], hat(z) = relu(1 - |z|), with
R = relative sample position; reference clipping semantics are reproduced
exactly by R = min(max(r, frac(r) - x), 255 - x), which differs from r
only at columns {0,1,254,255} (fixed with tiny border ops).

Slice layout on chip: [256,256] -> [128, 512] (partition p holds rows p and
p+128), padded to [128, 2*pad+512] so shifted-tap access patterns stay in
bounds.  Weights on ScalarE, products on VectorE, accumulation over slices
via TensorE identity-matmul into PSUM (start/stop accumulate flags).
"""
import numpy as np
import concourse.bacc as bacc
import concourse.bass as bass
import concourse.mybir as mybir
from concourse.tile import TileContext
from concourse.bass_utils import run_bass_kernel_spmd

EPS = 1e-3
BS, TS, TJ, H, W = 4, 64, 27, 256, 256
N_CORES = 8
TV = TS // 2          # voxel slices per core
JI = 14               # img slices per core (27 -> 14+13, half1 dup zeroed)
F = 512               # packed free dim: [128, 512] per [256,256] slice
FDT = mybir.dt.float32


def _pk(a):
    """[256,256] -> [128,512] packed: col blk*256+x on partition p = row blk*128+p."""
    return a.reshape(2, 128, 256).transpose(1, 0, 2).reshape(128, 512)


def _unpk(a):
    return a.reshape(128, 2, 256).transpose(1, 0, 2).reshape(256, 256)


def _dram_packed(t, i):
    """3-D AP for slice i of DRAM tensor t [N,256,256]: [p, blk, x]."""
    return t[i].rearrange("(blk p) x -> p blk x", blk=2)


def _sb_packed(tile_ap):
    """View a [128, 512] SBUF region as [p, blk, x]."""
    return tile_ap.rearrange("p (blk x) -> p blk x", blk=2)


def build(taps3: bool):
    nc = bacc.Bacc(None, target_bir_lowering=False, debug=False)
    dt = mybir.dt
    A = mybir.AluOpType
    AF = mybir.ActivationFunctionType

    for val in (-2.0, -1.0, 2.0):
        t = nc.alloc_sbuf_tensor(f"constx-{val}", [128, 1], mybir.dt.float32)
        nc.gpsimd.memset(t.ap(), val)
        nc.const_aps.aps[(mybir.dt.float32, val)] = t.ap()
    nc.all_engine_barrier()

    vox = nc.declare_dram_parameter("vox", [TV, H, W], FDT, isOutput=False)
    flowe = nc.declare_dram_parameter("flowe", [TV, H, W], FDT, isOutput=False)
    occ = nc.declare_dram_parameter("occ", [JI, H, W], FDT, isOutput=False)
    flowi = nc.declare_dram_parameter("flowi", [JI, H, W], FDT, isOutput=False)
    sfe = nc.declare_dram_parameter("sfe", [H, W], FDT, isOutput=False)
    sfi = nc.declare_dram_parameter("sfi", [H, W], FDT, isOutput=False)
    sdg = nc.declare_dram_parameter("sdg", [H, W], FDT, isOutput=False)
    # scal columns: [0:TV) -s_ev | [TV:TV+JI) -s_img | [TV+JI:TV+2JI) k_img gain
    #               | TV+2JI k_ev | TV+2JI+1 k_imgsingle | TV+2JI+2 g_gt
    NS = TV + 2 * JI + 3
    scal = nc.declare_dram_parameter("scal", [128, NS], FDT, isOutput=False)

    ov = nc.declare_dram_parameter("ov", [128, F], FDT, isOutput=True)
    oi = nc.declare_dram_parameter("oi", [128, F], FDT, isOutput=True)
    od = nc.declare_dram_parameter("od", [128, F], FDT, isOutput=True)
    oev = nc.declare_dram_parameter("oev", [128, F], FDT, isOutput=True)
    oiv = nc.declare_dram_parameter("oiv", [128, F], FDT, isOutput=True)
    ogt = nc.declare_dram_parameter("ogt", [128, F], FDT, isOutput=True)

    PAD = 1 if taps3 else 2
    FP = F + 2 * PAD          # padded tile width
    ds = (-1, 0, 1) if taps3 else (-2, -1, 0, 1, 2)

    with TileContext(nc) as tc:
        with tc.tile_pool(name="const", bufs=1) as cpool, \
             tc.tile_pool(name="io", bufs=3) as iop, \
             tc.tile_pool(name="wk", bufs=2) as wk, \
             tc.tile_pool(name="ps", bufs=1, space="PSUM") as psp:

            st = cpool.tile([128, NS], FDT, tag="st")
            nc.sync.dma_start(out=st[:], in_=scal[:])
            ident = cpool.tile([128, 128], FDT, tag="ident")
            iotap = cpool.tile([128, 1], FDT, tag="iotap")
            iotaf = cpool.tile([128, 128], FDT, tag="iotaf")
            nc.gpsimd.iota(iotap[:], pattern=[[0, 1]], channel_multiplier=1,
                           allow_small_or_imprecise_dtypes=True)
            nc.gpsimd.iota(iotaf[:], pattern=[[1, 128]], channel_multiplier=0,
                           allow_small_or_imprecise_dtypes=True)
            nc.vector.tensor_scalar(ident[:], iotaf[:], iotap[:, 0:1], None, A.is_equal)

            # border x-values tile: [0,1,0,1,0]; left AP -> [0,1,0,1], right AP -> [1,0,1,0]
            cb = cpool.tile([128, 5], FDT, tag="cb")
            nc.gpsimd.memset(cb[:], 0.0)
            nc.gpsimd.memset(cb[:, 1:4:2], 1.0)

            psv = psp.tile([128, F], FDT, tag="psv")
            psi = psp.tile([128, F], FDT, tag="psi")
            psd = psp.tile([128, F], FDT, tag="psd")

            def border_fix(r):
                """In-place: r <- min(max(r, frac(r)-x), 255-x) on border cols."""
                rc = r[:, PAD:PAD + F].rearrange("p (blk x) -> p blk x", blk=2)
                rl = rc[:, :, 0:2]
                rr = rc[:, :, 254:256]
                cbl = cb[:, 0:4].rearrange("p (blk x) -> p blk x", blk=2)   # x
                cbr = cb[:, 1:5].rearrange("p (blk x) -> p blk x", blk=2)   # 255-x
                ib = wk.tile([128, 2, 2], dt.int32, tag="ib")
                fb = wk.tile([128, 2, 2], FDT, tag="fb")
                wb = wk.tile([128, 2, 2], FDT, tag="wb")
                nc.vector.tensor_scalar(fb[:], rl, 0.5, None, A.subtract)
                nc.vector.tensor_copy(ib[:], fb[:])          # rint -> floor(r)
                nc.vector.tensor_copy(fb[:], ib[:])
                nc.vector.tensor_tensor(wb[:], rl, fb[:], A.subtract)   # frac
                nc.vector.tensor_tensor(wb[:], wb[:], cbl, A.subtract)  # - x
                nc.vector.tensor_tensor(rl, rl, wb[:], A.max)
                nc.vector.tensor_tensor(rr, rr, cbr, A.min)

            def warp_mac(r, src, psum, gain_col, first, last):
                """psum += sum_d hat(r-d)*src_shift_d; optional per-slice gain on weights."""
                hs = {}
                if taps3:
                    z = wk.tile([128, F], FDT, tag="z")
                    nc.scalar.activation(z[:], r[:, PAD:PAD + F], AF.Abs)
                    h0 = wk.tile([128, F], FDT, tag="h0")
                    nc.scalar.activation(h0[:], z[:], AF.Copy, bias=1.0, scale=-1.0)
                    hs[0] = h0
                    hp = wk.tile([128, F], FDT, tag="hp")
                    nc.scalar.activation(hp[:], r[:, PAD:PAD + F], AF.Relu)
                    hs[1] = hp
                    hm = wk.tile([128, F], FDT, tag="hm")
                    nc.scalar.activation(hm[:], r[:, PAD:PAD + F], AF.Relu, scale=-1.0)
                    hs[-1] = hm
                else:
                    for d in ds:
                        z = wk.tile([128, F], FDT, tag=f"z{d}")
                        nc.scalar.activation(z[:], r[:, PAD:PAD + F], AF.Abs, bias=float(-d))
                        h = wk.tile([128, F], FDT, tag=f"h{d}")
                        nc.scalar.activation(h[:], z[:], AF.Relu, bias=1.0, scale=-1.0)
                        hs[d] = h
                for k, d in enumerate(ds):
                    p = wk.tile([128, F], FDT, tag=f"p{d}")
                    nc.vector.tensor_tensor(p[:], hs[d][:], src[:, PAD + d:PAD + d + F],
                                            A.mult)
                    nc.tensor.matmul(psum[:], ident[:], p[:],
                                     start=(first and k == 0), stop=(last and k == len(ds) - 1))

            # ---------------- voxel stream ----------------
            for t in range(TV):
                ft = iop.tile([128, FP], FDT, tag="ft")
                nc.sync.dma_start(out=_sb_packed(ft[:, PAD:PAD + F]), in_=_dram_packed(flowe, t))
                vt = iop.tile([128, FP], FDT, tag="vt")
                nc.sync.dma_start(out=_sb_packed(vt[:, PAD:PAD + F]), in_=_dram_packed(vox, t))
                # finite pads (weights there are exactly 0)
                nc.gpsimd.memset(vt[:, 0:PAD], 0.0)
                nc.gpsimd.memset(vt[:, PAD + F:], 0.0)

                r = wk.tile([128, FP], FDT, tag="r")
                nc.vector.tensor_scalar(r[:, PAD:PAD + F], ft[:, PAD:PAD + F],
                                        EPS, st[:, t:t + 1], A.add, A.mult)
                border_fix(r)
                warp_mac(r, vt, psv, None, first=(t == 0), last=(t == TV - 1))

            # ---------------- img + depth stream ----------------
            for j in range(JI):
                ft = iop.tile([128, FP], FDT, tag="ft")
                nc.sync.dma_start(out=_sb_packed(ft[:, PAD:PAD + F]), in_=_dram_packed(flowi, j))
                ot = iop.tile([128, FP], FDT, tag="vt")
                nc.sync.dma_start(out=_sb_packed(ot[:, PAD:PAD + F]), in_=_dram_packed(occ, j))
                nc.gpsimd.memset(ot[:, 0:PAD], 0.0)
                nc.gpsimd.memset(ot[:, PAD + F:], 0.0)

                fp = wk.tile([128, FP], FDT, tag="fp")
                nc.vector.tensor_scalar(fp[:, PAD:PAD + F], ft[:, PAD:PAD + F],
                                        EPS, None, A.add)
                r = wk.tile([128, FP], FDT, tag="r")
                nc.vector.tensor_scalar(r[:, PAD:PAD + F], fp[:, PAD:PAD + F],
                                        st[:, TV + j:TV + j + 1], None, A.mult)
                border_fix(r)

                dep = wk.tile([128, FP], FDT, tag="dep")
                nc.vector.reciprocal(dep[:, PAD:PAD + F], fp[:, PAD:PAD + F])
                nc.vector.tensor_scalar(dep[:, PAD:PAD + F], dep[:, PAD:PAD + F],
                                        st[:, TV + JI + j:TV + JI + j + 1], None, A.mult)
                nc.gpsimd.memset(dep[:, 0:PAD], 0.0)
                nc.gpsimd.memset(dep[:, PAD + F:], 0.0)

                warp_mac(r, ot, psi, None, first=(j == 0), last=(j == JI - 1))
                warp_mac(r, dep, psd, None, first=(j == 0), last=(j == JI - 1))

            # ---------------- singles ----------------
            def single_recip(src_dram, gain_col, out_dram):
                t_in = iop.tile([128, F], FDT, tag="sing")
                nc.sync.dma_start(out=_sb_packed(t_in[:]),
                                  in_=src_dram.rearrange("(blk p) x -> p blk x", blk=2))
                t2 = wk.tile([128, F], FDT, tag="sing2")
                nc.vector.tensor_scalar(t2[:], t_in[:], EPS, None, A.add)
                nc.vector.reciprocal(t2[:], t2[:])
                nc.vector.tensor_scalar(t2[:], t2[:], st[:, gain_col:gain_col + 1],
                                        None, A.mult)
                nc.sync.dma_start(out=out_dram[:], in_=t2[:])

            single_recip(sfe, TV + 2 * JI, oev)
            single_recip(sfi, TV + 2 * JI + 1, oiv)
            tgt = iop.tile([128, F], FDT, tag="sing")
            nc.sync.dma_start(out=_sb_packed(tgt[:]),
                              in_=sdg.rearrange("(blk p) x -> p blk x", blk=2))
            tg2 = wk.tile([128, F], FDT, tag="sing2")
            nc.vector.tensor_scalar(tg2[:], tgt[:], st[:, TV + 2 * JI + 2:TV + 2 * JI + 3],
                                    None, A.mult)
            nc.sync.dma_start(out=ogt[:], in_=tg2[:])

            # ---------------- psum -> out ----------------
            for psum, out_dram, scale in ((psv, ov, 1.0 / TS), (psi, oi, 1.0 / TJ),
                                          (psd, od, 1.0 / TJ)):
                o = wk.tile([128, F], FDT, tag="ocp")
                nc.scalar.activation(o[:], psum[:], AF.Copy, bias=0.0, scale=scale)
                nc.sync.dma_start(out=out_dram[:], in_=o[:])

    nc.finalize()
    return nc


_CACHED = {}
_RUNNERS = {}
LAST_EXEC_NS = None


def _build_runner(nc, n_cores=N_CORES):
    """Compiled SPMD callable mirroring bass2jax.run_bass_via_pjrt (no donation)."""
    import jax
    import numpy as _np
    from jax.sharding import Mesh, PartitionSpec
    try:
        from jax.experimental.shard_map import shard_map
    except ImportError:
        from jax.shard_map import shard_map
    from concourse import bass2jax, mybir as _mybir

    bass2jax.install_neuronx_cc_hook()
    partition_name = nc.partition_id_tensor.name if nc.partition_id_tensor else None
    in_names, out_names, out_avals, zero_outs = [], [], [], []
    for alloc in nc.m.functions[0].allocations:
        if not isinstance(alloc, _mybir.MemoryLocationSet):
            continue
        name = alloc.memorylocations[0].name
        if alloc.kind == "ExternalInput":
            if name != partition_name:
                in_names.append(name)
        elif alloc.kind == "ExternalOutput":
            shape = tuple(alloc.tensor_shape)
            dtype = _mybir.dt.np(alloc.dtype)
            out_names.append(name)
            out_avals.append(jax.core.ShapedArray(shape, dtype))
            zero_outs.append(_np.zeros(shape, dtype))
    n_params = len(in_names)
    all_in_names = in_names + out_names
    if partition_name is not None:
        all_in_names = all_in_names + [partition_name]

    def _body(*args):
        operands = list(args)
        if partition_name is not None:
            operands.append(bass2jax.partition_id_tensor())
        outs = bass2jax._bass_exec_p.bind(
            *operands,
            out_avals=tuple(out_avals),
            in_names=tuple(all_in_names),
            out_names=tuple(out_names),
            lowering_input_output_aliases=(),
            sim_require_finite=True,
            sim_require_nnan=True,
            nc=nc,
        )
        return tuple(outs)

    devices = jax.devices()[:n_cores]
    mesh = Mesh(np.asarray(devices), ("core",))
    in_specs = (PartitionSpec("core"),) * (n_params + len(out_names))
    out_specs = (PartitionSpec("core"),) * len(out_names)
    sharded = jax.jit(shard_map(_body, mesh=mesh, in_specs=in_specs,
                                out_specs=out_specs, check_rep=False))

    def run(in_maps, time_iters=0):
        concat_in = [np.concatenate([np.asarray(m[name]) for m in in_maps], axis=0)
                     for name in in_names]
        concat_zeros = [np.concatenate([z] * n_cores, axis=0) for z in zero_outs]
        sh = jax.sharding.NamedSharding(mesh, PartitionSpec("core"))
        dev_args = [jax.device_put(a, sh) for a in concat_in + concat_zeros]
        outs = sharded(*dev_args)
        jax.block_until_ready(outs)
        exec_ns = None
        if time_iters:
            import time as _t
            best = float("inf")
            for _ in range(time_iters):
                t0 = _t.perf_counter()
                outs = sharded(*dev_args)
                jax.block_until_ready(outs)
                best = min(best, _t.perf_counter() - t0)
            exec_ns = int(best * 1e9)
        host_outs = [np.asarray(o) for o in outs]
        results = []
        for c in range(n_cores):
            d = {}
            for name, arr in zip(out_names, host_outs):
                per = arr.shape[0] // n_cores
                d[name] = arr[c * per:(c + 1) * per]
            results.append(d)
        return results, exec_ns

    return run


def _get_nc(taps3: bool):
    if taps3 not in _CACHED:
        _CACHED[taps3] = build(taps3)
    return _CACHED[taps3]


def kernel(voxelgrid, time, occ_aps, occ_t, gt_t, fx, v, depth_gt, flow_27):
    voxelgrid = np.asarray(voxelgrid, dtype=np.float32)
    time = np.asarray(time, dtype=np.float32)
    occ_aps = np.asarray(occ_aps, dtype=np.float32)
    occ_t = np.asarray(occ_t, dtype=np.float32)
    gt_t = np.asarray(gt_t, dtype=np.float32)
    fx = np.asarray(fx, dtype=np.float32)
    v = np.asarray(v, dtype=np.float32)
    depth_gt = np.asarray(depth_gt, dtype=np.float32)
    flow_27 = np.asarray(flow_27, dtype=np.float32)

    s_ev = time - gt_t[:, None]                     # [4,64]
    s_img = occ_t - gt_t[:, None]                   # [4,27]
    k = fx[:, 0, 0] * np.abs(v)                     # [4] depth numerator
    dist = np.abs(occ_t[:, None, :] - time[:, :, None])
    idx = np.argmin(dist, axis=2)                   # [4,64]
    ev_idx = np.argmin(np.abs(s_ev), axis=1)        # [4]
    img_idx = np.argmin(np.abs(s_img), axis=1)      # [4]

    taps3 = float(np.max(np.abs(np.concatenate([s_ev.ravel(), s_img.ravel()])))) \
        * (1.0 + EPS) < 1.0

    NS = TV + 2 * JI + 3
    in_maps = []
    for c in range(N_CORES):
        b, half = c // 2, c % 2
        tlo = half * TV
        tsl = slice(tlo, tlo + TV)
        jlist = list(range(0, JI)) if half == 0 else list(range(JI, TJ)) + [TJ - 1]
        jdup = [False] * JI if half == 0 else [False] * (TJ - JI) + [True]

        vox_s = voxelgrid[b, tsl]                                   # [32,H,W]
        flowe_s = flow_27[b, idx[b, tlo:tlo + TV]]                  # [32,H,W]
        occ_s = np.stack([np.zeros((H, W), np.float32) if dup else occ_aps[b, j]
                          for j, dup in zip(jlist, jdup)])
        flowi_s = flow_27[b, jlist]

        scal = np.zeros((128, NS), np.float32)
        scal[:, 0:TV] = -s_ev[b, tsl][None, :]
        scal[:, TV:TV + JI] = -s_img[b, jlist][None, :]
        scal[:, TV + JI:TV + 2 * JI] = np.where(jdup, 0.0, k[b])[None, :]

        own_ev = (tlo <= ev_idx[b] < tlo + TV)
        own_img = img_idx[b] in [j for j, dup in zip(jlist, jdup) if not dup]
        sfe_s = flow_27[b, idx[b, ev_idx[b]]] if own_ev else np.ones((H, W), np.float32)
        sfi_s = flow_27[b, img_idx[b]] if own_img else np.ones((H, W), np.float32)
        sdg_s = depth_gt[b, img_idx[b]] if own_img else np.zeros((H, W), np.float32)
        scal[:, TV + 2 * JI] = k[b] if own_ev else 0.0
        scal[:, TV + 2 * JI + 1] = k[b] if own_img else 0.0
        scal[:, TV + 2 * JI + 2] = 1.0 if own_img else 0.0

        in_maps.append({
            "vox": np.ascontiguousarray(vox_s),
            "flowe": np.ascontiguousarray(flowe_s),
            "occ": np.ascontiguousarray(occ_s),
            "flowi": np.ascontiguousarray(flowi_s),
            "sfe": np.ascontiguousarray(sfe_s),
            "sfi": np.ascontiguousarray(sfi_s),
            "sdg": np.ascontiguousarray(sdg_s),
            "scal": scal,
        })

    import os
    nc = _get_nc(taps3)
    if taps3 not in _RUNNERS:
        _RUNNERS[taps3] = _build_runner(nc)
    iters = int(os.environ.get("KERNEL_TIME_ITERS", "0"))
    results, exec_ns = _RUNNERS[taps3](in_maps, time_iters=iters)
    global LAST_EXEC_NS
    LAST_EXEC_NS = exec_ns

    class _Res:
        pass
    res = _Res()
    res.results = results

    out = np.zeros((BS, 6, H, W), np.float32)
    for b in range(BS):
        r0, r1 = res.results[2 * b], res.results[2 * b + 1]
        out[b, 0] = _unpk(r0["ov"] + r1["ov"])
        out[b, 1] = _unpk(r0["oi"] + r1["oi"])
        out[b, 2] = _unpk(r0["od"] + r1["od"])
        out[b, 3] = _unpk(r0["oev"] + r1["oev"])
        out[b, 4] = _unpk(r0["oiv"] + r1["oiv"])
        out[b, 5] = _unpk(r0["ogt"] + r1["ogt"])
    return out
